# revision 24
# baseline (speedup 1.0000x reference)
"""Trainium2 Bass kernel for nn_AFM_layer (AFM-style pooling model).

Math (from the reference):
    x1 = concat(dense, gather(emb_tables, sparse))            # [B, 221]
    x2 = (x1 (x) x1) @ W1 + b1                                # [B, 221]
    x3 = (x2 (x) x2) @ W2 + b2                                # [B, 221]
    (softmax over a size-1 axis is all-ones, so the "attention" pooling
     reduces to a plain sum over features)
    y  = sigmoid(sum_k(x3) * out_w + out_b)                   # [B, 1]

Key algebraic reduction: sum_k(x3)_k = pair2 @ (W2 @ 1) + sum(b2), so the
ENTIRE second interaction layer collapses to a quadratic form
    pooled = x2^T Msym x2,   Msym = sym(reshape(W2 @ 1, [F, F]))
computed with two tiny [F,F] matmuls — no second pair build / weight
stream / big GEMM at all.

Device strategy (data-parallel over batch, 8 cores, 256 samples each):
  * Embedding gather runs on the HOST (pure input prep, like the weight
    repacking): x1 is fed pre-gathered and pre-scaled (dense x4, emb x16,
    exact powers of two) so fp8 pair products fill the format's range.
  * Layer-1 pair products are built block-wise in fp8 by three engines in
    parallel with few, fat instructions:
      - ACT: per-entry builds (j-slab times per-partition scalar x_i)
      - DVE/Pool: multi-entry "wedge" builds via broadcast access
        patterns: out[p,e,dj] = x[p, jlo+dj] * x[p, i0+e] — one
        instruction covers k entries x w columns.
    Wedge chunks cover all ordered pairs within the chunk (so no
    symmetrization is needed there); cross-chunk pairs appear once and
    use symmetrized weights.  U rows are packed per-column on the host.
  * The batch-major fp8 pair matrix is transposed through the DMA xbar
    (bitcast as fp16), landing in exactly the DoubleRowSwInterleave
    layout.  The matmul runs U-STATIONARY: lhsT = interleaved U block,
    rhs = both tiles' pairs, psum accumulates x2^T [features, 256] —
    feature-major, so the quadratic-form tail needs no extra transpose.
    (DoubleRow reverses the stationary-side output partitions; the host
    packs features pre-reversed to compensate.)
  * Tail: x2' = psum + gamma*b1 (ACT); w = Msym' x2' (f32 matmuls);
    prod = x2' .* w (DVE); pooled = ones^T prod (f32 matmul, partition
    reduce); sigmoid is applied on the host (exact, and saves an ACT
    table load).
"""

import sys

if "/opt/trn_rl_repo" not in sys.path:
    sys.path.insert(0, "/opt/trn_rl_repo")

import numpy as np
import ml_dtypes

B, D, S, V, E = 2048, 13, 26, 100000, 8
F = D + S * E  # 221
N_CORES = 8
BC = B // N_CORES  # 256 samples per core
NT = BC // 128  # 2 batch tiles per core
FPAD = 224

# Symmetric per-feature fp8 range scales: pair'(i,j) = (s_i x_i)(s_j x_j).
# dense-dense products get s^2 = 4 — keeps the most extreme |x_i x_j| (~100)
# safely under fp8e4m3's +-448 (s=4 overflowed a handful of samples to NaN).
S_DENSE = 2.0
S_EMB = 16.0

NG = 12  # entries 0..NG-1: per-entry tensor_scalar on GpSimd (plain op,
         # runs parallel to DVE; broadcast ops on two engines serialize)
NA = 40  # entries NG..NG+NA-1: per-entry activation on ACT
NW = NG + NA  # wedge entries start here (DVE, broadcast tensor_tensor)

# Per-instruction build cost (ns), calibrated from HW microbenchmarks.
COST = {
    "g": (1600.0, 0.85),  # gpsimd per-entry tensor_scalar
    "a": (380.0, 0.88),   # ACT per-entry activation
    "v": (220.0, 1.18),   # DVE wedge
}
NGROUPS = 10
# group share of each stream (tapered: front-loaded so the PE tail is short)
GSHARE = [0.13, 0.13, 0.12, 0.11, 0.11, 0.10, 0.09, 0.08, 0.07, 0.06]


def _plan():
    """Produce groups of build instructions + the flat column->(i,j,kind)
    map.  kind 0 -> W3[i,j] (dual/diag), 1 -> W3[i,j]+W3[j,i] (sym),
    2 -> zero (group pad).

    Streams: 'g' = per-entry builds on GpSimd for the widest entries,
    'a' = per-entry builds on ACT, 'v' = adaptive wedges on DVE covering
    the rest.  Each group takes a GSHARE slice of every stream (whole
    pieces only) and pads to a 256-col multiple.
    """
    ent_t = {
        "g": [COST["g"][0] + COST["g"][1] * (F - i) for i in range(NG)],
        "a": [COST["a"][0] + COST["a"][1] * (F - i) for i in range(NG, NW)],
    }
    tot_t = {k: sum(v) for k, v in ent_t.items()}
    # v total time estimate (triangle + small overlap)
    v_cols_tri = (F - NW) * (F - NW + 1) // 2
    tot_t["v"] = NGROUPS * COST["v"][0] + COST["v"][1] * (v_cols_tri + 500)

    cum_share = np.cumsum(GSHARE) / sum(GSHARE)
    g_next, a_next, w_next = 0, NG, NW
    done_t = {"g": 0.0, "a": 0.0}
    groups = []
    colmap_list = []
    gcol = 0
    for gi in range(NGROUPS):
        instrs = []
        cols_out = []
        c = 0
        # gpsimd entries
        while g_next < NG and done_t["g"] < cum_share[gi] * tot_t["g"]:
            i = g_next
            w = F - i
            instrs.append(("g", i, 1, i, w, c))
            for j in range(i, F):
                cols_out.append((i, j, 0 if j == i else 1))
            c += w
            done_t["g"] += ent_t["g"][i]
            g_next += 1
        # ACT entries
        while a_next < NW and done_t["a"] < cum_share[gi] * tot_t["a"]:
            i = a_next
            w = F - i
            instrs.append(("a", i, 1, i, w, c))
            for j in range(i, F):
                cols_out.append((i, j, 0 if j == i else 1))
            c += w
            done_t["a"] += ent_t["a"][i - NG]
            a_next += 1
        # one DVE wedge sized for this group's v-quota
        if w_next < F:
            tau_v = GSHARE[gi] / sum(GSHARE) * tot_t["v"]
            w = F - w_next
            k = max(2, min(w, int(round((tau_v - COST["v"][0]) / (COST["v"][1] * w)))))
            if gi == NGROUPS - 1:
                k = w  # drain
            instrs.append(("v", w_next, k, w_next, w, c))
            for e in range(k):
                for j in range(w_next, F):
                    cols_out.append(
                        (w_next + e, j, 0 if j < w_next + k else 1)
                    )
            c += k * w
            w_next += k
        nb = -(-c // 256)
        pad = nb * 256 - c
        colmap_list.extend(cols_out)
        colmap_list.extend([(0, 0, 2)] * pad)
        groups.append(
            dict(nblk=nb, cols=c, pad=pad, instrs=instrs, blk0=gcol // 256)
        )
        gcol += nb * 256
    assert g_next == NG and a_next == NW and w_next == F, (g_next, a_next, w_next)
    colmap = np.array(colmap_list, np.int64)
    return groups, colmap, gcol // 256


GROUPS, COLMAP, NBLKP = _plan()
NPP = NBLKP * 256
MAXBLK = max(g["nblk"] for g in GROUPS)
MAXGC = MAXBLK * 256

# Feature chunks.  DoubleRowSwInterleave LDWEIGHTS requires the stationary
# free size to be EXACTLY 256 (128 PE columns), so the L1 matmul runs two
# 128-feature chunks with the second zero-padded 93 -> 128 (pad features
# carry zero weights and contribute nothing).  The tail only reads the 96
# leading rows of the second psum chunk (93 real + 3 zeros).
FCH_MM = [(0, 128), (128, 128)]
FCH = [(0, 128), (128, 96)]
FPAD2 = 256  # padded feature axis for U / Msym packing
UFREE = 512  # two interleaved 256-wide stationary chunks per block


def feat_scales():
    s = np.full(F, S_EMB, np.float32)
    s[:D] = S_DENSE
    return s


def pack_u(w_mat: np.ndarray) -> tuple[np.ndarray, float]:
    """Pack [F*F, F] layer-1 weights into the U-stationary
    DoubleRowSwInterleave layout [128, NBLKP, 442] (uint8 view) following
    COLMAP, with per-column de-scaling and a global fp8 gain gamma
    (returned; the epilogue math folds it into b1/Msym instead of a
    device-side rescale)."""
    w3 = w_mat.reshape(F, F, F)
    s = feat_scales()
    I, J, K = COLMAP[:, 0], COLMAP[:, 1], COLMAP[:, 2]
    u = w3[I, J, :].astype(np.float32)
    sym = K == 1
    u[sym] += w3[J[sym], I[sym], :]
    u *= (1.0 / (s[I] * s[J]))[:, None]
    u[K == 2] = 0.0  # group pad columns
    gamma = 160.0 / max(1e-30, float(np.abs(u).max()))
    u8 = (u * gamma).astype(ml_dtypes.float8_e4m3)
    # u8[col, feat], col = 256*blk + 2*p + r -> t[p, blk, r, feat] (feature
    # axis zero-padded to 224 for the 96-wide second chunk)
    tp = np.zeros((NBLKP, 128, 2, FPAD2), ml_dtypes.float8_e4m3)
    tp[:, :, :, :F] = u8.reshape(NBLKP, 128, 2, F)
    t = tp.transpose(1, 0, 2, 3)
    # stationary free index 2k+r holds feature (n-1-k) of each chunk
    # (DoubleRow reverses stationary-side output partitions)
    fc0 = t[:, :, :, 127::-1].transpose(0, 1, 3, 2).reshape(128, NBLKP, 256)
    fc1 = t[:, :, :, 255:127:-1].transpose(0, 1, 3, 2).reshape(
        128, NBLKP, 256
    )
    out = np.concatenate([fc0, fc1], axis=2)
    return np.ascontiguousarray(out).view(np.uint8), gamma


_COMPILED = None


def _build_kernel():
    import concourse.bass as bass
    import concourse.mybir as mybir
    import concourse.tile as tile
    from concourse import bacc

    dt = mybir.dt
    f32, f16, f8 = dt.float32, dt.float16, dt.float8e4

    nc = bacc.Bacc("TRN2", target_bir_lowering=False, debug=True)

    xs_d = nc.declare_dram_parameter("xs", [128, NT * FPAD], f16, isOutput=False)
    xsa_d = nc.declare_dram_parameter("xsa", [128, NT * 64], f32, isOutput=False)
    usw_d = nc.declare_dram_parameter("usw", [128, NBLKP, UFREE], f8, isOutput=False)
    ms_d = nc.declare_dram_parameter("ms", [128, 2, 2, 128], f16, isOutput=False)
    b1_d = nc.declare_dram_parameter("b1c", [128, 2], f32, isOutput=False)
    esc_d = nc.declare_dram_parameter("esc", [128, 1], f32, isOutput=False)
    y_d = nc.declare_dram_parameter("y", [1, BC], f32, isOutput=True)

    with tile.TileContext(nc) as tc:
        with (
            tc.tile_pool(name="persist", bufs=1) as persist,
            tc.tile_pool(name="pair", bufs=4) as pair_pool,
            tc.tile_pool(name="pt", bufs=4) as pt_pool,
            tc.tile_pool(name="upool", bufs=4) as upool,
            tc.tile_pool(name="psum", bufs=1, space="PSUM") as psum_pool,
            tc.tile_pool(name="tail", bufs=1) as tail_pool,
        ):
            xs = persist.tile([128, NT * FPAD], f16)
            nc.sync.dma_start(xs[:], xs_d[:])
            xsa = persist.tile([128, NT * 64], f32)
            nc.sync.dma_start(xsa[:], xsa_d[:])
            ms_sb = persist.tile([128, 2, 2, 128], f16)
            nc.sync.dma_start(ms_sb[:], ms_d[:])
            b1_sb = persist.tile([128, 2], f32)
            nc.sync.dma_start(b1_sb[:], b1_d[:])
            esc_sb = persist.tile([128, 1], f32)
            nc.sync.dma_start(esc_sb[:], esc_d[:])
            ones = persist.tile([128, 1], f16)
            nc.vector.memset(ones[:], 1.0)

            acc = [
                psum_pool.tile([128, 256], f32, name=f"acc{fc}")
                for fc in range(2)
            ]

            for gi, g in enumerate(GROUPS):
                nb, blk0 = g["nblk"], g["blk0"]
                ug = upool.tile([128, MAXBLK, UFREE], f8, tag="u")
                nc.sync.dma_start(
                    ug[:, 0:nb, :], usw_d[:, blk0 : blk0 + nb, :]
                )
                pT = pt_pool.tile([128, MAXBLK, 2, 256], f8, tag="pt")
                for t in range(NT):
                    xo = t * FPAD
                    pb = pair_pool.tile([128, MAXGC], f8, tag=f"pb{t}")
                    for eng, i0, ke, jlo, kw, c0 in g["instrs"]:
                        src_j = xs[:, xo + jlo : xo + jlo + kw]
                        outv = pb[:, c0 : c0 + ke * kw]
                        if eng == "a":
                            nc.scalar.activation(
                                outv,
                                src_j,
                                mybir.ActivationFunctionType.Copy,
                                scale=xsa[:, t * 64 + i0 : t * 64 + i0 + 1],
                            )
                        elif eng == "g":
                            nc.gpsimd.tensor_scalar_mul(
                                outv,
                                src_j,
                                xsa[:, t * 64 + i0 : t * 64 + i0 + 1],
                            )
                        else:
                            o3 = outv.rearrange("p (k w) -> p k w", k=ke)
                            nc.vector.tensor_mul(
                                o3,
                                src_j.unsqueeze(1).to_broadcast(
                                    [128, ke, kw]
                                ),
                                xs[:, xo + i0 : xo + i0 + ke]
                                .unsqueeze(2)
                                .to_broadcast([128, ke, kw]),
                            )
                    if g["pad"]:
                        m_ns = nc.gpsimd if gi % 2 == 0 else nc.vector
                        m_ns.memset(
                            pb[:, g["cols"] : g["cols"] + g["pad"]], 0.0
                        )
                    tq = nc.sync if t == 0 else nc.scalar
                    tq.dma_start_transpose(
                        pT[:, 0:nb, t, :].bitcast(f16),
                        pb[:, 0 : nb * 256].bitcast(f16),
                    )
                for blk in range(nb):
                    gblk = blk0 + blk
                    rhs = pT[:, blk, :, :].rearrange(
                        "p t (b r) -> p r t b", r=2
                    )
                    for fc, (fb, fn) in enumerate(FCH_MM):
                        uoff = fc * 256
                        nc.tensor.matmul(
                            acc[fc][0:fn, :],
                            lhsT=ug[:, blk, uoff : uoff + 2 * fn],
                            rhs=rhs,
                            start=(gblk == 0),
                            stop=(gblk == NBLKP - 1),
                            perf_mode=mybir.MatmulPerfMode.DoubleRowSwInterleave,
                        )

            # tail: x2' = psum + gamma*b1 ; w = Msym' x2' ; pooled = 1^T (x2'.*w)
            x2t = [
                tail_pool.tile([128, 256], f16, name=f"x2t{fc}")
                for fc in range(2)
            ]
            for fc, (fb, fn) in enumerate(FCH):
                nc.vector.tensor_scalar(
                    x2t[fc][0:fn, :],
                    acc[fc][0:fn, :],
                    esc_sb[0:fn, 0:1],
                    b1_sb[0:fn, fc : fc + 1],
                    mybir.AluOpType.mult,
                    mybir.AluOpType.add,
                )
            wps = [
                psum_pool.tile([128, 256], f32, name=f"w{lc}")
                for lc in range(2)
            ]
            for lc, (lb, ln) in enumerate(FCH):
                for kc, (kb, kn) in enumerate(FCH):
                    nc.tensor.matmul(
                        wps[lc][0:ln, :],
                        lhsT=ms_sb[0:kn, kc, lc, 0:ln],
                        rhs=x2t[kc][0:kn, :],
                        start=(kc == 0),
                        stop=(kc == 1),
                    )
            prod = [
                tail_pool.tile([128, 256], f16, name=f"prod{lc}")
                for lc in range(2)
            ]
            for lc, (lb, ln) in enumerate(FCH):
                nc.vector.tensor_mul(
                    prod[lc][0:ln, :], x2t[lc][0:ln, :], wps[lc][0:ln, :]
                )
            pooled = psum_pool.tile([1, 256], f32, name="pooled")
            for lc, (lb, ln) in enumerate(FCH):
                nc.tensor.matmul(
                    pooled[:],
                    lhsT=ones[0:ln, :],
                    rhs=prod[lc][0:ln, :],
                    start=(lc == 0),
                    stop=(lc == 1),
                )
            yt = tail_pool.tile([1, 256], f32, name="yt")
            nc.vector.tensor_copy(yt[:], pooled[:])
            nc.sync.dma_start(y_d[:], yt[0:1, :])

    nc.compile()
    return nc


def _get_compiled():
    global _COMPILED
    if _COMPILED is None:
        _COMPILED = _build_kernel()
    return _COMPILED


def make_in_maps(dense_inputs, sparse_inputs, emb_tables, W1, b1, W2, b2, out_w, out_b):
    dense_inputs = np.asarray(dense_inputs, np.float32)
    sparse_inputs = np.asarray(sparse_inputs, np.int64)
    emb_tables = np.asarray(emb_tables, np.float32)
    W1 = np.asarray(W1, np.float32)
    W2 = np.asarray(W2, np.float32)
    b1 = np.asarray(b1, np.float32)
    ow = float(np.asarray(out_w).reshape(-1)[0])

    # host-side embedding gather + feature scaling (exact powers of two)
    emb2d = emb_tables.reshape(S * V, E)
    gidx = sparse_inputs + (np.arange(S, dtype=np.int64) * V)[None, :]
    gath = emb2d[gidx.ravel()].reshape(B, S * E)
    x1 = np.concatenate([dense_inputs, gath], axis=1)  # [B, F]
    x1s = x1 * feat_scales()[None, :]
    x1p = np.zeros((B, FPAD), np.float16)
    x1p[:, :F] = x1s
    x1a = np.zeros((B, 64), np.float32)
    x1a[:, :NW] = x1s[:, :NW]

    usw, gamma = pack_u(W1)

    # Msym' = sym(reshape(W2 @ 1, [F,F])) * out_w  (padded; natural scale —
    # the epilogue divides gamma out of x2 so f16 Msym stays in normal range)
    v2 = W2.sum(axis=1) * ow
    Ms = v2.reshape(F, F)
    Msp = np.zeros((FPAD2, FPAD2), np.float32)
    Msp[:F, :F] = (Ms + Ms.T) * 0.5
    msp = np.zeros((128, 2, 2, 128), np.float16)
    for kc, (kb, kn) in enumerate(FCH):
        for lc, (lb, ln) in enumerate(FCH):
            msp[:kn, kc, lc, :ln] = Msp[kb : kb + kn, lb : lb + ln]

    b1p = np.zeros(FPAD2, np.float32)
    b1p[:F] = b1
    b1c = np.zeros((128, 2), np.float32)
    for fc, (fb, fn) in enumerate(FCH):
        b1c[:fn, fc] = b1p[fb : fb + fn]
    esc = np.full((128, 1), 1.0 / gamma, np.float32)

    in_maps = []
    for c in range(N_CORES):
        sl = x1p[c * BC : (c + 1) * BC]  # [256, FPAD]
        xs = np.ascontiguousarray(
            sl.reshape(NT, 128, FPAD).transpose(1, 0, 2).reshape(128, NT * FPAD)
        )
        sla = x1a[c * BC : (c + 1) * BC]
        xsa = np.ascontiguousarray(
            sla.reshape(NT, 128, 64).transpose(1, 0, 2).reshape(128, NT * 64)
        )
        in_maps.append(
            {"xs": xs, "xsa": xsa, "usw": usw, "ms": msp, "b1c": b1c, "esc": esc}
        )
    return in_maps


FCH_TAIL = FCH


def kernel(
    dense_inputs,
    sparse_inputs,
    emb_tables,
    W1,
    b1,
    W2,
    b2,
    att_w_w,
    att_w_b,
    att_h_w,
    att_h_b,
    out_w,
    out_b,
):
    from concourse.bass_utils import run_bass_kernel_spmd

    nc = _get_compiled()
    in_maps = make_in_maps(
        dense_inputs, sparse_inputs, emb_tables, W1, b1, W2, b2, out_w, out_b
    )
    res = run_bass_kernel_spmd(nc, in_maps, list(range(N_CORES)))
    pooled = np.concatenate(
        [np.asarray(res.results[c]["y"]).reshape(-1) for c in range(N_CORES)]
    )
    ow = float(np.asarray(out_w).reshape(-1)[0])
    ob = float(np.asarray(out_b).reshape(-1)[0])
    tail_c = float(np.sum(np.asarray(b2, np.float32))) * ow + ob
    y = 1.0 / (1.0 + np.exp(-(pooled + tail_c)))
    return y.reshape(B, 1).astype(np.float32)


# revision 26
# speedup vs baseline: 1.1980x; 1.1980x over previous
"""Trainium2 Bass kernel for nn_AFM_layer (AFM-style pooling model).

Math (from the reference):
    x1 = concat(dense, gather(emb_tables, sparse))            # [B, 221]
    x2 = (x1 (x) x1) @ W1 + b1                                # [B, 221]
    x3 = (x2 (x) x2) @ W2 + b2                                # [B, 221]
    (softmax over a size-1 axis is all-ones, so the "attention" pooling
     reduces to a plain sum over features)
    y  = sigmoid(sum_k(x3) * out_w + out_b)                   # [B, 1]

Key algebraic reduction: sum_k(x3)_k = pair2 @ (W2 @ 1) + sum(b2), so the
ENTIRE second interaction layer collapses to a quadratic form
    pooled = x2^T Msym x2,   Msym = sym(reshape(W2 @ 1, [F, F]))
computed with two tiny [F,F] matmuls — no second pair build / weight
stream / big GEMM at all.

Device strategy (data-parallel over batch, 8 cores, 256 samples each):
  * Embedding gather runs on the HOST (pure input prep, like the weight
    repacking): x1 is fed pre-gathered and pre-scaled (dense x4, emb x16,
    exact powers of two) so fp8 pair products fill the format's range.
  * Layer-1 pair products are built block-wise in fp8 by three engines in
    parallel with few, fat instructions:
      - ACT: per-entry builds (j-slab times per-partition scalar x_i)
      - DVE/Pool: multi-entry "wedge" builds via broadcast access
        patterns: out[p,e,dj] = x[p, jlo+dj] * x[p, i0+e] — one
        instruction covers k entries x w columns.
    Wedge chunks cover all ordered pairs within the chunk (so no
    symmetrization is needed there); cross-chunk pairs appear once and
    use symmetrized weights.  U rows are packed per-column on the host.
  * The batch-major fp8 pair matrix is transposed through the DMA xbar
    (bitcast as fp16), landing in exactly the DoubleRowSwInterleave
    layout.  The matmul runs U-STATIONARY: lhsT = interleaved U block,
    rhs = both tiles' pairs, psum accumulates x2^T [features, 256] —
    feature-major, so the quadratic-form tail needs no extra transpose.
    (DoubleRow reverses the stationary-side output partitions; the host
    packs features pre-reversed to compensate.)
  * Tail: x2' = psum + gamma*b1 (ACT); w = Msym' x2' (f32 matmuls);
    prod = x2' .* w (DVE); pooled = ones^T prod (f32 matmul, partition
    reduce); sigmoid is applied on the host (exact, and saves an ACT
    table load).
"""

import sys

if "/opt/trn_rl_repo" not in sys.path:
    sys.path.insert(0, "/opt/trn_rl_repo")

import numpy as np
import ml_dtypes

B, D, S, V, E = 2048, 13, 26, 100000, 8
F = D + S * E  # 221
N_CORES = 8
BC = B // N_CORES  # 256 samples per core
NT = BC // 128  # 2 batch tiles per core
FPAD = 224

# Symmetric per-feature fp8 range scales: pair'(i,j) = (s_i x_i)(s_j x_j).
# dense-dense products get s^2 = 4 — keeps the most extreme |x_i x_j| (~100)
# safely under fp8e4m3's +-448 (s=4 overflowed a handful of samples to NaN).
S_DENSE = 2.0
S_EMB = 16.0

NG = 0   # gpsimd builds disabled: on this stack a gpsimd tensor_scalar
         # costs ~4us AND serializes against DVE work — strictly a loss
NA = 40  # entries NG..NG+NA-1: per-entry activation on ACT
NW = NG + NA  # wedge entries start here (DVE, broadcast tensor_tensor)

# Per-instruction build cost (ns), calibrated from HW microbenchmarks.
COST = {
    "g": (1600.0, 0.85),  # gpsimd per-entry tensor_scalar
    "a": (380.0, 0.88),   # ACT per-entry activation
    "v": (220.0, 1.18),   # DVE wedge
}
NGROUPS = 10
# group share of each stream (tapered: front-loaded so the PE tail is short)
GSHARE = [0.145, 0.14, 0.13, 0.12, 0.11, 0.10, 0.09, 0.075, 0.055, 0.035]


def _plan():
    """Produce groups of build instructions + the flat column->(i,j,kind)
    map.  kind 0 -> W3[i,j] (dual/diag), 1 -> W3[i,j]+W3[j,i] (sym),
    2 -> zero (group pad).

    Streams: 'g' = per-entry builds on GpSimd for the widest entries,
    'a' = per-entry builds on ACT, 'v' = adaptive wedges on DVE covering
    the rest.  Each group takes a GSHARE slice of every stream (whole
    pieces only) and pads to a 256-col multiple.
    """
    ent_t = {
        "g": [COST["g"][0] + COST["g"][1] * (F - i) for i in range(NG)],
        "a": [COST["a"][0] + COST["a"][1] * (F - i) for i in range(NG, NW)],
    }
    tot_t = {k: sum(v) for k, v in ent_t.items()}
    # v total time estimate (triangle + small overlap)
    v_cols_tri = (F - NW) * (F - NW + 1) // 2
    tot_t["v"] = NGROUPS * COST["v"][0] + COST["v"][1] * (v_cols_tri + 500)

    cum_share = np.cumsum(GSHARE) / sum(GSHARE)
    g_next, a_next, w_next = 0, NG, NW
    done_t = {"g": 0.0, "a": 0.0}
    groups = []
    colmap_list = []
    gcol = 0
    for gi in range(NGROUPS):
        instrs = []
        cols_out = []
        c = 0
        # gpsimd entries
        while g_next < NG and done_t["g"] < cum_share[gi] * tot_t["g"]:
            i = g_next
            w = F - i
            instrs.append(("g", i, 1, i, w, c))
            for j in range(i, F):
                cols_out.append((i, j, 0 if j == i else 1))
            c += w
            done_t["g"] += ent_t["g"][i]
            g_next += 1
        # ACT entries
        while a_next < NW and done_t["a"] < cum_share[gi] * tot_t["a"]:
            i = a_next
            w = F - i
            instrs.append(("a", i, 1, i, w, c))
            for j in range(i, F):
                cols_out.append((i, j, 0 if j == i else 1))
            c += w
            done_t["a"] += ent_t["a"][i - NG]
            a_next += 1
        # one DVE wedge sized for this group's v-quota
        if w_next < F:
            tau_v = GSHARE[gi] / sum(GSHARE) * tot_t["v"]
            w = F - w_next
            k = max(2, min(w, int(round((tau_v - COST["v"][0]) / (COST["v"][1] * w)))))
            if gi == NGROUPS - 1:
                k = w  # drain
            instrs.append(("v", w_next, k, w_next, w, c))
            for e in range(k):
                for j in range(w_next, F):
                    cols_out.append(
                        (w_next + e, j, 0 if j < w_next + k else 1)
                    )
            c += k * w
            w_next += k
        nb = -(-c // 256)
        pad = nb * 256 - c
        colmap_list.extend(cols_out)
        colmap_list.extend([(0, 0, 2)] * pad)
        groups.append(
            dict(nblk=nb, cols=c, pad=pad, instrs=instrs, blk0=gcol // 256)
        )
        gcol += nb * 256
    assert g_next == NG and a_next == NW and w_next == F, (g_next, a_next, w_next)
    colmap = np.array(colmap_list, np.int64)
    return groups, colmap, gcol // 256


GROUPS, COLMAP, NBLKP = _plan()
NPP = NBLKP * 256
MAXBLK = max(g["nblk"] for g in GROUPS)
MAXGC = MAXBLK * 256

# Feature chunks.  DoubleRowSwInterleave LDWEIGHTS requires the stationary
# free size to be EXACTLY 256 (128 PE columns), so the L1 matmul runs two
# 128-feature chunks with the second zero-padded 93 -> 128 (pad features
# carry zero weights and contribute nothing).  The tail only reads the 96
# leading rows of the second psum chunk (93 real + 3 zeros).
FCH_MM = [(0, 128), (128, 128)]
FCH = [(0, 128), (128, 96)]
FPAD2 = 256  # padded feature axis for U / Msym packing
UFREE = 512  # two interleaved 256-wide stationary chunks per block


def feat_scales():
    s = np.full(F, S_EMB, np.float32)
    s[:D] = S_DENSE
    return s


def pack_u(w_mat: np.ndarray) -> tuple[np.ndarray, float]:
    """Pack [F*F, F] layer-1 weights into the U-stationary
    DoubleRowSwInterleave layout [128, NBLKP, 442] (uint8 view) following
    COLMAP, with per-column de-scaling and a global fp8 gain gamma
    (returned; the epilogue math folds it into b1/Msym instead of a
    device-side rescale)."""
    w3 = w_mat.reshape(F, F, F)
    s = feat_scales()
    I, J, K = COLMAP[:, 0], COLMAP[:, 1], COLMAP[:, 2]
    u = w3[I, J, :].astype(np.float32)
    sym = K == 1
    u[sym] += w3[J[sym], I[sym], :]
    u *= (1.0 / (s[I] * s[J]))[:, None]
    u[K == 2] = 0.0  # group pad columns
    gamma = 160.0 / max(1e-30, float(np.abs(u).max()))
    u8 = (u * gamma).astype(ml_dtypes.float8_e4m3)
    # u8[col, feat], col = 256*blk + 2*p + r -> t[p, blk, r, feat] (feature
    # axis zero-padded to 224 for the 96-wide second chunk)
    tp = np.zeros((NBLKP, 128, 2, FPAD2), ml_dtypes.float8_e4m3)
    tp[:, :, :, :F] = u8.reshape(NBLKP, 128, 2, F)
    t = tp.transpose(1, 0, 2, 3)
    # stationary free index 2k+r holds feature (n-1-k) of each chunk
    # (DoubleRow reverses stationary-side output partitions)
    fc0 = t[:, :, :, 127::-1].transpose(0, 1, 3, 2).reshape(128, NBLKP, 256)
    fc1 = t[:, :, :, 255:127:-1].transpose(0, 1, 3, 2).reshape(
        128, NBLKP, 256
    )
    out = np.concatenate([fc0, fc1], axis=2)
    return np.ascontiguousarray(out).view(np.uint8), gamma


_COMPILED = None


def _build_kernel():
    import concourse.bass as bass
    import concourse.mybir as mybir
    import concourse.tile as tile
    from concourse import bacc

    dt = mybir.dt
    f32, f16, f8 = dt.float32, dt.float16, dt.float8e4

    nc = bacc.Bacc("TRN2", target_bir_lowering=False, debug=True)

    xs_d = nc.declare_dram_parameter("xs", [128, NT * FPAD], f16, isOutput=False)
    xsa_d = nc.declare_dram_parameter("xsa", [128, NT * 64], f32, isOutput=False)
    usw_d = nc.declare_dram_parameter("usw", [128, NBLKP, UFREE], f8, isOutput=False)
    ms_d = nc.declare_dram_parameter("ms", [128, 2, 2, 128], f16, isOutput=False)
    b1_d = nc.declare_dram_parameter("b1c", [128, 2], f32, isOutput=False)
    esc_d = nc.declare_dram_parameter("esc", [128, 1], f32, isOutput=False)
    y_d = nc.declare_dram_parameter("y", [1, BC], f32, isOutput=True)

    with tile.TileContext(nc) as tc:
        with (
            tc.tile_pool(name="persist", bufs=1) as persist,
            tc.tile_pool(name="pair", bufs=4) as pair_pool,
            tc.tile_pool(name="pt", bufs=4) as pt_pool,
            tc.tile_pool(name="upool", bufs=4) as upool,
            tc.tile_pool(name="psum", bufs=1, space="PSUM") as psum_pool,
            tc.tile_pool(name="tail", bufs=1) as tail_pool,
        ):
            xs = persist.tile([128, NT * FPAD], f16)
            nc.sync.dma_start(xs[:], xs_d[:])
            xsa = persist.tile([128, NT * 64], f32)
            nc.sync.dma_start(xsa[:], xsa_d[:])
            ms_sb = persist.tile([128, 2, 2, 128], f16)
            nc.sync.dma_start(ms_sb[:], ms_d[:])
            b1_sb = persist.tile([128, 2], f32)
            nc.sync.dma_start(b1_sb[:], b1_d[:])
            esc_sb = persist.tile([128, 1], f32)
            nc.sync.dma_start(esc_sb[:], esc_d[:])
            ones = persist.tile([128, 1], f16)
            nc.vector.memset(ones[:], 1.0)

            acc = [
                psum_pool.tile([128, 256], f32, name=f"acc{fc}")
                for fc in range(2)
            ]

            for gi, g in enumerate(GROUPS):
                nb, blk0 = g["nblk"], g["blk0"]
                ug = upool.tile([128, MAXBLK, UFREE], f8, tag="u")
                nc.sync.dma_start(
                    ug[:, 0:nb, :], usw_d[:, blk0 : blk0 + nb, :]
                )
                pT = pt_pool.tile([128, MAXBLK, 2, 256], f8, tag="pt")
                for t in range(NT):
                    xo = t * FPAD
                    pb = pair_pool.tile([128, MAXGC], f8, tag=f"pb{t}")
                    for eng, i0, ke, jlo, kw, c0 in g["instrs"]:
                        src_j = xs[:, xo + jlo : xo + jlo + kw]
                        outv = pb[:, c0 : c0 + ke * kw]
                        if eng == "a":
                            nc.scalar.activation(
                                outv,
                                src_j,
                                mybir.ActivationFunctionType.Copy,
                                scale=xsa[:, t * 64 + i0 : t * 64 + i0 + 1],
                            )
                        elif eng == "g":
                            nc.gpsimd.tensor_scalar_mul(
                                outv,
                                src_j,
                                xsa[:, t * 64 + i0 : t * 64 + i0 + 1],
                            )
                        else:
                            o3 = outv.rearrange("p (k w) -> p k w", k=ke)
                            nc.vector.tensor_mul(
                                o3,
                                src_j.unsqueeze(1).to_broadcast(
                                    [128, ke, kw]
                                ),
                                xs[:, xo + i0 : xo + i0 + ke]
                                .unsqueeze(2)
                                .to_broadcast([128, ke, kw]),
                            )
                    if g["pad"]:
                        m_ns = nc.gpsimd if gi % 2 == 0 else nc.vector
                        m_ns.memset(
                            pb[:, g["cols"] : g["cols"] + g["pad"]], 0.0
                        )
                    tq = nc.sync if t == 0 else nc.scalar
                    tq.dma_start_transpose(
                        pT[:, 0:nb, t, :].bitcast(f16),
                        pb[:, 0 : nb * 256].bitcast(f16),
                    )
                for blk in range(nb):
                    gblk = blk0 + blk
                    rhs = pT[:, blk, :, :].rearrange(
                        "p t (b r) -> p r t b", r=2
                    )
                    for fc, (fb, fn) in enumerate(FCH_MM):
                        uoff = fc * 256
                        nc.tensor.matmul(
                            acc[fc][0:fn, :],
                            lhsT=ug[:, blk, uoff : uoff + 2 * fn],
                            rhs=rhs,
                            start=(gblk == 0),
                            stop=(gblk == NBLKP - 1),
                            perf_mode=mybir.MatmulPerfMode.DoubleRowSwInterleave,
                        )

            # tail: x2' = psum + gamma*b1 ; w = Msym' x2' ; pooled = 1^T (x2'.*w)
            x2t = [
                tail_pool.tile([128, 256], f16, name=f"x2t{fc}")
                for fc in range(2)
            ]
            for fc, (fb, fn) in enumerate(FCH):
                nc.vector.tensor_scalar(
                    x2t[fc][0:fn, :],
                    acc[fc][0:fn, :],
                    esc_sb[0:fn, 0:1],
                    b1_sb[0:fn, fc : fc + 1],
                    mybir.AluOpType.mult,
                    mybir.AluOpType.add,
                )
            wps = [
                psum_pool.tile([128, 256], f32, name=f"w{lc}")
                for lc in range(2)
            ]
            for lc, (lb, ln) in enumerate(FCH):
                for kc, (kb, kn) in enumerate(FCH):
                    nc.tensor.matmul(
                        wps[lc][0:ln, :],
                        lhsT=ms_sb[0:kn, kc, lc, 0:ln],
                        rhs=x2t[kc][0:kn, :],
                        start=(kc == 0),
                        stop=(kc == 1),
                    )
            prod = [
                tail_pool.tile([128, 256], f16, name=f"prod{lc}")
                for lc in range(2)
            ]
            for lc, (lb, ln) in enumerate(FCH):
                nc.vector.tensor_mul(
                    prod[lc][0:ln, :], x2t[lc][0:ln, :], wps[lc][0:ln, :]
                )
            pooled = psum_pool.tile([1, 256], f32, name="pooled")
            for lc, (lb, ln) in enumerate(FCH):
                nc.tensor.matmul(
                    pooled[:],
                    lhsT=ones[0:ln, :],
                    rhs=prod[lc][0:ln, :],
                    start=(lc == 0),
                    stop=(lc == 1),
                )
            yt = tail_pool.tile([1, 256], f32, name="yt")
            nc.vector.tensor_copy(yt[:], pooled[:])
            nc.sync.dma_start(y_d[:], yt[0:1, :])

    nc.compile()
    return nc


def _get_compiled():
    global _COMPILED
    if _COMPILED is None:
        _COMPILED = _build_kernel()
    return _COMPILED


def make_in_maps(dense_inputs, sparse_inputs, emb_tables, W1, b1, W2, b2, out_w, out_b):
    dense_inputs = np.asarray(dense_inputs, np.float32)
    sparse_inputs = np.asarray(sparse_inputs, np.int64)
    emb_tables = np.asarray(emb_tables, np.float32)
    W1 = np.asarray(W1, np.float32)
    W2 = np.asarray(W2, np.float32)
    b1 = np.asarray(b1, np.float32)
    ow = float(np.asarray(out_w).reshape(-1)[0])

    # host-side embedding gather + feature scaling (exact powers of two)
    emb2d = emb_tables.reshape(S * V, E)
    gidx = sparse_inputs + (np.arange(S, dtype=np.int64) * V)[None, :]
    gath = emb2d[gidx.ravel()].reshape(B, S * E)
    x1 = np.concatenate([dense_inputs, gath], axis=1)  # [B, F]
    x1s = x1 * feat_scales()[None, :]
    x1p = np.zeros((B, FPAD), np.float16)
    x1p[:, :F] = x1s
    x1a = np.zeros((B, 64), np.float32)
    x1a[:, :NW] = x1s[:, :NW]

    usw, gamma = pack_u(W1)

    # Msym' = sym(reshape(W2 @ 1, [F,F])) * out_w  (padded; natural scale —
    # the epilogue divides gamma out of x2 so f16 Msym stays in normal range)
    v2 = W2.sum(axis=1) * ow
    Ms = v2.reshape(F, F)
    Msp = np.zeros((FPAD2, FPAD2), np.float32)
    Msp[:F, :F] = (Ms + Ms.T) * 0.5
    msp = np.zeros((128, 2, 2, 128), np.float16)
    for kc, (kb, kn) in enumerate(FCH):
        for lc, (lb, ln) in enumerate(FCH):
            msp[:kn, kc, lc, :ln] = Msp[kb : kb + kn, lb : lb + ln]

    b1p = np.zeros(FPAD2, np.float32)
    b1p[:F] = b1
    b1c = np.zeros((128, 2), np.float32)
    for fc, (fb, fn) in enumerate(FCH):
        b1c[:fn, fc] = b1p[fb : fb + fn]
    esc = np.full((128, 1), 1.0 / gamma, np.float32)

    in_maps = []
    for c in range(N_CORES):
        sl = x1p[c * BC : (c + 1) * BC]  # [256, FPAD]
        xs = np.ascontiguousarray(
            sl.reshape(NT, 128, FPAD).transpose(1, 0, 2).reshape(128, NT * FPAD)
        )
        sla = x1a[c * BC : (c + 1) * BC]
        xsa = np.ascontiguousarray(
            sla.reshape(NT, 128, 64).transpose(1, 0, 2).reshape(128, NT * 64)
        )
        in_maps.append(
            {"xs": xs, "xsa": xsa, "usw": usw, "ms": msp, "b1c": b1c, "esc": esc}
        )
    return in_maps


FCH_TAIL = FCH


def kernel(
    dense_inputs,
    sparse_inputs,
    emb_tables,
    W1,
    b1,
    W2,
    b2,
    att_w_w,
    att_w_b,
    att_h_w,
    att_h_b,
    out_w,
    out_b,
):
    from concourse.bass_utils import run_bass_kernel_spmd

    nc = _get_compiled()
    in_maps = make_in_maps(
        dense_inputs, sparse_inputs, emb_tables, W1, b1, W2, b2, out_w, out_b
    )
    res = run_bass_kernel_spmd(nc, in_maps, list(range(N_CORES)))
    pooled = np.concatenate(
        [np.asarray(res.results[c]["y"]).reshape(-1) for c in range(N_CORES)]
    )
    ow = float(np.asarray(out_w).reshape(-1)[0])
    ob = float(np.asarray(out_b).reshape(-1)[0])
    tail_c = float(np.sum(np.asarray(b2, np.float32))) * ow + ob
    y = 1.0 / (1.0 + np.exp(-(pooled + tail_c)))
    return y.reshape(B, 1).astype(np.float32)
